# revision 2
# baseline (speedup 1.0000x reference)
"""Llama4TextExperts MoE kernel for 8 Trainium2 NeuronCores.

Expert-parallel: core e handles expert e (tokens pre-sorted per expert).
Per core: x_e (1024,2048) @ gate_up[e] (2048,8192) -> silu(gate)*up ->
@ down[e] (4096,2048) -> out_e (1024,2048).

Device layout puts features on partitions (tokens on the free dim), so
the host supplies x transposed per expert and transposes the output
back. Phase 1 (gate/up projection) runs in fp32r (FP22 multiply, fp32
accumulate, full PE rate at N=512). The activations are stored bf16 in
SBUF (8 MB instead of 16 MB, which is what lets everything stay
resident); phase 2 (down projection) runs in bf16 with fp32 PSUM
accumulation.
"""

import numpy as np
import ml_dtypes

NUM_EXPERTS = 8
HIDDEN = 2048
INTER = 4096
TOKENS = 8192
T = TOKENS // NUM_EXPERTS  # 1024 tokens per expert/core
TK = HIDDEN // 128  # 16 contraction tiles in phase 1
TI = INTER // 128  # 32 feature tiles of gate/up; contraction tiles in phase 2
TH = HIDDEN // 128  # 16 output feature tiles
NT = T // 512  # 2 token chunks of 512


def _split_waits(nc, max_waits=1):
    """The walrus build in this environment rejects instructions carrying
    more than one sync wait. Move excess SyncWaits onto preceding NoOps
    on the same engine (semantically identical: the engine stalls on the
    NoOps first)."""
    import concourse.mybir as mybir

    for fn in nc.m.functions:
        for blk in fn.blocks:
            new_insts = []
            for inst in blk.instructions:
                si = inst.sync_info
                if si is not None and len(si.on_wait) > max_waits:
                    waits = list(si.on_wait)
                    excess, keep = waits[:-max_waits], waits[-max_waits:]
                    for i in range(0, len(excess), max_waits):
                        chunk = excess[i : i + max_waits]
                        new_insts.append(
                            mybir.InstNoOp(
                                name=f"{inst.name}-waitsplit-{i}",
                                ins=[],
                                outs=[],
                                engine=inst.engine,
                                sync_info=mybir.SyncInfo(
                                    on_wait=list(chunk), on_update=[]
                                ),
                            )
                        )
                    si.on_wait = keep
                new_insts.append(inst)
            blk.instructions = new_insts


def build_bass():
    import concourse.bass as bass
    import concourse.mybir as mybir
    import concourse.tile as tile

    F32 = mybir.dt.float32
    F32R = mybir.dt.float32r
    BF16 = mybir.dt.bfloat16
    Silu = mybir.ActivationFunctionType.Silu

    nc = bass.Bass()
    xT = nc.declare_dram_parameter("xT", [HIDDEN, T], F32R, isOutput=False)
    w1 = nc.declare_dram_parameter("w1", [HIDDEN, 2 * INTER], F32R, isOutput=False)
    w2 = nc.declare_dram_parameter("w2", [INTER, HIDDEN], BF16, isOutput=False)
    outT = nc.declare_dram_parameter("outT", [HIDDEN, T], F32, isOutput=True)

    xT_t = xT.rearrange("(kk p) t -> kk p t", p=128)
    outT_t = outT.rearrange("(hh p) t -> hh p t", p=128)

    with tile.TileContext(nc) as tc:
        with tc.tile_pool(name="acted", bufs=1) as actp:
            actts = [actp.tile([128, T], BF16, tag=f"a{i}", name=f"acted{i}") for i in range(TI)]

            # ---- Phase 1: gate/up projection + silu(gate)*up -> acted ----
            with tc.tile_pool(name="xres", bufs=1) as xp, \
                 tc.tile_pool(name="w1s", bufs=3) as w1p, \
                 tc.tile_pool(name="tmp", bufs=4) as tmpp, \
                 tc.tile_pool(name="ps1", bufs=2, space="PSUM") as ps1:
                xts = [xp.tile([128, T], F32R, tag=f"x{k}", name=f"xres{k}") for k in range(TK)]
                for k in range(TK):
                    nc.sync.dma_start(out=xts[k], in_=xT_t[k])

                for g in range(TI):
                    wg = w1p.tile([128, TK, 128], F32R, tag="wg")
                    wu = w1p.tile([128, TK, 128], F32R, tag="wu")
                    c = g * 128
                    nc.sync.dma_start(
                        out=wg,
                        in_=w1[:, c : c + 128].rearrange("(kk p) f -> p kk f", p=128),
                    )
                    nc.sync.dma_start(
                        out=wu,
                        in_=w1[:, INTER + c : INTER + c + 128].rearrange(
                            "(kk p) f -> p kk f", p=128
                        ),
                    )
                    pg = [ps1.tile([128, 512], F32, tag=f"pg{t}", name=f"psg{g}_{t}") for t in range(NT)]
                    pu = [ps1.tile([128, 512], F32, tag=f"pu{t}", name=f"psu{g}_{t}") for t in range(NT)]
                    for kk in range(TK):
                        st, sp = kk == 0, kk == TK - 1
                        for t in range(NT):
                            ts = slice(t * 512, (t + 1) * 512)
                            nc.tensor.matmul(
                                pg[t], wg[:, kk, :], xts[kk][:, ts], start=st, stop=sp
                            )
                            nc.tensor.matmul(
                                pu[t], wu[:, kk, :], xts[kk][:, ts], start=st, stop=sp
                            )
                    for t in range(NT):
                        ts = slice(t * 512, (t + 1) * 512)
                        sg = tmpp.tile([128, 512], F32, tag="sg")
                        nc.scalar.activation(sg, pg[t], Silu)
                        nc.vector.tensor_mul(actts[g][:, ts], sg, pu[t])

            # ---- Phase 2: down projection -> outT ----
            with tc.tile_pool(name="w2s", bufs=3) as w2p, \
                 tc.tile_pool(name="outs", bufs=3) as outp, \
                 tc.tile_pool(name="ps2", bufs=2, space="PSUM") as ps2:
                for h in range(TH):
                    w2h = w2p.tile([128, TI, 128], BF16, tag="w2")
                    c = h * 128
                    nc.sync.dma_start(
                        out=w2h,
                        in_=w2[:, c : c + 128].rearrange("(ii p) f -> p ii f", p=128),
                    )
                    po = [ps2.tile([128, 512], F32, tag=f"po{t}", name=f"pso{h}_{t}") for t in range(NT)]
                    for ii in range(TI):
                        st, sp = ii == 0, ii == TI - 1
                        for t in range(NT):
                            ts = slice(t * 512, (t + 1) * 512)
                            nc.tensor.matmul(
                                po[t], w2h[:, ii, :], actts[ii][:, ts], start=st, stop=sp
                            )
                    ot = outp.tile([128, T], F32, tag="ot")
                    for t in range(NT):
                        ts = slice(t * 512, (t + 1) * 512)
                        nc.vector.tensor_copy(ot[:, ts], po[t])
                    nc.sync.dma_start(out=outT_t[h], in_=ot)

    _split_waits(nc, 1)
    return nc


def make_in_maps(hidden_states, gate_up_proj, down_proj):
    x = np.asarray(hidden_states, dtype=np.float32).reshape(NUM_EXPERTS, T, HIDDEN)
    w1 = np.asarray(gate_up_proj, dtype=np.float32)
    w2 = np.asarray(down_proj).astype(ml_dtypes.bfloat16)
    in_maps = []
    for e in range(NUM_EXPERTS):
        in_maps.append(
            {
                "xT": np.ascontiguousarray(x[e].T),
                "w1": np.ascontiguousarray(w1[e]),
                "w2": np.ascontiguousarray(w2[e]),
            }
        )
    return in_maps


def assemble_output(results):
    outs = [results[e]["outT"].T for e in range(NUM_EXPERTS)]
    return np.concatenate(outs, axis=0).astype(np.float32)


def kernel(hidden_states, gate_up_proj, down_proj):
    from concourse.bass_utils import run_bass_kernel_spmd

    nc = build_bass()
    in_maps = make_in_maps(hidden_states, gate_up_proj, down_proj)
    res = run_bass_kernel_spmd(nc, in_maps, list(range(NUM_EXPERTS)))
    return assemble_output(res.results)


# revision 10
# speedup vs baseline: 81.7556x; 81.7556x over previous
"""Llama4TextExperts MoE kernel for 8 Trainium2 NeuronCores.

Expert-parallel: core e handles expert e (tokens pre-sorted per expert).
Per core: x_e (1024,2048) @ gate_up[e] (2048,8192) -> silu(gate)*up ->
@ down[e] (4096,2048) -> out_e (1024,2048).

Device layout puts features on partitions (tokens on the free dim): the
host supplies x transposed per expert and transposes the output back.
Weights are host-reordered tile-major so every weight-tile DMA is one
fully contiguous block (strided column gathers measured ~2x slower).

Phase 1 (gate/up projection) runs in fp32r: FP22 multiply at full PE
rate with the 4-byte weight load fused into the matmul instruction
(fp32r matmuls are self-loading; separate LDWEIGHTS is not legal for
them), accumulating fp32 in PSUM. The activations silu(gate)*up are
stored fp16 in SBUF (8 MB, which is what lets x + acted stay resident);
phase 2 (down projection) runs in fp16 with fp32 PSUM accumulation.
Measured end-to-end relative error vs the fp32 reference: ~4e-4.
"""

import numpy as np

NUM_EXPERTS = 8
HIDDEN = 2048
INTER = 4096
TOKENS = 8192
T = TOKENS // NUM_EXPERTS  # 1024 tokens per expert/core
TK = HIDDEN // 128  # 16 contraction tiles in phase 1
TI = INTER // 128  # 32 feature tiles of gate/up; contraction tiles in phase 2
TH = HIDDEN // 128  # 16 output feature tiles
NT = T // 512  # 2 token chunks of 512


def _split_waits(nc, max_waits=1):
    """The walrus build in this environment rejects instructions carrying
    more than one sync wait. Move excess SyncWaits onto preceding NoOps
    on the same engine (semantically identical: the engine stalls on the
    NoOps first)."""
    import concourse.mybir as mybir

    for fn in nc.m.functions:
        for blk in fn.blocks:
            new_insts = []
            for inst in blk.instructions:
                si = inst.sync_info
                if si is not None and len(si.on_wait) > max_waits:
                    waits = list(si.on_wait)
                    excess, keep = waits[:-max_waits], waits[-max_waits:]
                    for i in range(0, len(excess), max_waits):
                        chunk = excess[i : i + max_waits]
                        new_insts.append(
                            mybir.InstNoOp(
                                name=f"{inst.name}-waitsplit-{i}",
                                ins=[],
                                outs=[],
                                engine=inst.engine,
                                sync_info=mybir.SyncInfo(
                                    on_wait=list(chunk), on_update=[]
                                ),
                            )
                        )
                    si.on_wait = keep
                new_insts.append(inst)
            blk.instructions = new_insts


def build_bass(repeat=1):
    import contextlib

    import concourse.bass as bass
    import concourse.mybir as mybir
    import concourse.tile as tile

    F32 = mybir.dt.float32
    F32R = mybir.dt.float32r
    F16 = mybir.dt.float16
    Silu = mybir.ActivationFunctionType.Silu

    nc = bass.Bass()
    xT = nc.declare_dram_parameter("xT", [HIDDEN, T], F32R, isOutput=False)
    # host-reordered tile-major: w1[g, p, kk, f] = gate_up[kk*128+p, g*128+f]
    # (g 0..31 = gate blocks, 32..63 = up blocks); w2[h, p, ii, f] = down[ii*128+p, h*128+f]
    w1 = nc.declare_dram_parameter("w1", [2 * TI, 128, TK, 128], F32R, isOutput=False)
    w2 = nc.declare_dram_parameter("w2", [TH, 128, TI, 128], F16, isOutput=False)
    outT = nc.declare_dram_parameter("outT", [HIDDEN, T], F32, isOutput=True)

    xT_t = xT.rearrange("(kk p) t -> kk p t", p=128)
    outT_t = outT.rearrange("(hh p) t -> hh p t", p=128)

    with tile.TileContext(nc) as tc:
        rep = tc.For_i(0, repeat, 1) if repeat > 1 else contextlib.nullcontext()
        with rep, tc.tile_pool(name="acted", bufs=1) as actp:
            actts = [
                actp.tile([128, T], F16, tag=f"a{i}", name=f"acted{i}")
                for i in range(TI)
            ]

            # ---- Phase 1: gate/up projection + silu(gate)*up -> acted ----
            with tc.tile_pool(name="xres", bufs=1) as xp, \
                 tc.tile_pool(name="w1s", bufs=3) as w1p, \
                 tc.tile_pool(name="tmp", bufs=4) as tmpp, \
                 tc.tile_pool(name="ps1", bufs=2, space="PSUM") as ps1:
                xts = [
                    xp.tile([128, T], F32R, tag=f"x{k}", name=f"xres{k}")
                    for k in range(TK)
                ]
                for k in range(TK):
                    nc.sync.dma_start(out=xts[k], in_=xT_t[k])

                for g in range(TI):
                    wg = w1p.tile([128, TK, 128], F32R, tag="wg")
                    wu = w1p.tile([128, TK, 128], F32R, tag="wu")
                    nc.sync.dma_start(out=wg, in_=w1[g])
                    nc.sync.dma_start(out=wu, in_=w1[TI + g])
                    pg = [
                        ps1.tile([128, 512], F32, tag=f"pg{t}", name=f"psg{g}_{t}")
                        for t in range(NT)
                    ]
                    pu = [
                        ps1.tile([128, 512], F32, tag=f"pu{t}", name=f"psu{g}_{t}")
                        for t in range(NT)
                    ]
                    for kk in range(TK):
                        st, sp = kk == 0, kk == TK - 1
                        for t in range(NT):
                            ts = slice(t * 512, (t + 1) * 512)
                            nc.tensor.matmul(
                                pg[t], wg[:, kk, :], xts[kk][:, ts], start=st, stop=sp
                            )
                        for t in range(NT):
                            ts = slice(t * 512, (t + 1) * 512)
                            nc.tensor.matmul(
                                pu[t], wu[:, kk, :], xts[kk][:, ts], start=st, stop=sp
                            )
                    for t in range(NT):
                        ts = slice(t * 512, (t + 1) * 512)
                        sg = tmpp.tile([128, 512], F32, tag="sg")
                        nc.scalar.activation(sg, pg[t], Silu)
                        nc.vector.tensor_mul(actts[g][:, ts], sg, pu[t])

            # ---- Phase 2: down projection -> outT ----
            with tc.tile_pool(name="w2s", bufs=3) as w2p, \
                 tc.tile_pool(name="outs", bufs=3) as outp, \
                 tc.tile_pool(name="ps2", bufs=2, space="PSUM") as ps2:
                for h in range(TH):
                    w2h = w2p.tile([128, TI, 128], F16, tag="w2")
                    nc.sync.dma_start(out=w2h, in_=w2[h])
                    po = [
                        ps2.tile([128, 512], F32, tag=f"po{t}", name=f"pso{h}_{t}")
                        for t in range(NT)
                    ]
                    for ii in range(TI):
                        st, sp = ii == 0, ii == TI - 1
                        for t in range(NT):
                            ts = slice(t * 512, (t + 1) * 512)
                            nc.tensor.matmul(
                                po[t], w2h[:, ii, :], actts[ii][:, ts],
                                start=st, stop=sp,
                            )
                    ot = outp.tile([128, T], F32, tag="ot")
                    for t in range(NT):
                        ts = slice(t * 512, (t + 1) * 512)
                        nc.vector.tensor_copy(ot[:, ts], po[t])
                    nc.sync.dma_start(out=outT_t[h], in_=ot)

    _split_waits(nc, 1)
    return nc


def make_in_maps(hidden_states, gate_up_proj, down_proj):
    x = np.asarray(hidden_states, dtype=np.float32).reshape(NUM_EXPERTS, T, HIDDEN)
    w1 = np.asarray(gate_up_proj, dtype=np.float32)
    w2 = np.asarray(down_proj).astype(np.float16)
    in_maps = []
    for e in range(NUM_EXPERTS):
        # (H, 2I) -> (2I/128 g, 128 p, H/128 kk, 128 f) tile-major contiguous
        w1r = w1[e].reshape(TK, 128, 2 * TI, 128).transpose(2, 1, 0, 3)
        # (I, H) -> (H/128 h, 128 p, I/128 ii, 128 f)
        w2r = w2[e].reshape(TI, 128, TH, 128).transpose(2, 1, 0, 3)
        in_maps.append(
            {
                "xT": np.ascontiguousarray(x[e].T),
                "w1": np.ascontiguousarray(w1r),
                "w2": np.ascontiguousarray(w2r),
            }
        )
    return in_maps


def assemble_output(results):
    outs = [results[e]["outT"].T for e in range(NUM_EXPERTS)]
    return np.concatenate(outs, axis=0).astype(np.float32)


def kernel(hidden_states, gate_up_proj, down_proj):
    from concourse.bass_utils import run_bass_kernel_spmd

    nc = build_bass()
    in_maps = make_in_maps(hidden_states, gate_up_proj, down_proj)
    res = run_bass_kernel_spmd(nc, in_maps, list(range(NUM_EXPERTS)))
    return assemble_output(res.results)
